# revision 25
# baseline (speedup 1.0000x reference)
"""Trainium2 Bass kernel for nn_DenseGRU (8-core data-parallel over batch).

Math notes (exact algebraic simplifications of the reference):
  - The attention softmax is over a singleton axis -> weights == 1.0
    exactly, so ctx_t = x[:, :, t, :].sum(axis=1) and W1/b1/W2/b2/V/bV
    never affect the output.
  - gi_t = W_ih @ [ctx_t, gt] + b_ih is h-independent -> precomputed for
    all t in one GEMM before the scan. b_ih (and b_hh for the r/z gates)
    are folded into that GEMM via an extra all-ones input row.
  - Per-step recurrent work is only gh = W_hh @ h plus gate elementwise.

Layouts (per core = one batch sample):
  - h kept in "column" layout [128 partitions, 4] (h[c] at partition
    c%128, col c//128) so all gate elementwise ops use 128 partitions.
  - gh computed J-form: out[j-chunk partitions, 1] = W_hhT_chunk.T @ h_col,
    48 (ldweights+matmul) pairs per step.
"""

import sys
import types
import numpy as np
from contextlib import ExitStack

import concourse.bass as bass
import concourse.tile as tile
from concourse import bacc, mybir
from concourse.bass_utils import run_bass_kernel_spmd

F32 = mybir.dt.float32
F16 = mybir.dt.float16

B, HW, T, C = 8, 256, 32, 512
H, OUT = 512, 4
G3 = 3 * H          # 1536 gate rows
NJG = G3 // 128     # 12 j-groups
NKC = H // 128      # 4 c-chunks
TC = 8              # max timesteps per pipeline chunk
NCH = T // TC       # 4 chunks
CHS = [4, 4, 8, 8, 8]           # chunk sizes (sum = T)
CH0 = [0, 4, 8, 16, 24]         # chunk start timesteps

USE_F16 = True      # f16 weights/x for 2x LDWEIGHTS (FWL) + half DMA
DTW = F16 if USE_F16 else F32
NPW = np.float16 if USE_F16 else np.float32


def build_program():
    nc = bacc.Bacc("TRN2", target_bir_lowering=False, debug=False,
                   enable_asserts=False, num_devices=8)

    xb = nc.dram_tensor("xb", [HW, T, C], DTW, kind="ExternalInput")
    wih = nc.dram_tensor("wih", [C + OUT + 1, G3], DTW, kind="ExternalInput")
    whh = nc.dram_tensor("whh", [H, G3], DTW, kind="ExternalInput")
    aug = nc.dram_tensor("aug", [OUT + 1, T], DTW, kind="ExternalInput")
    bhn = nc.dram_tensor("bhn", [NKC, 128], DTW, kind="ExternalInput")
    id4 = nc.dram_tensor("id4", [NKC, NKC], DTW, kind="ExternalInput")
    wfc = nc.dram_tensor("wfc", [128, NKC * (OUT + 1)], F32, kind="ExternalInput")
    bfc = nc.dram_tensor("bfc", [1, OUT + 1], F32, kind="ExternalInput")
    oh = nc.dram_tensor("oh", [128, T], F32, kind="ExternalInput")
    outp = nc.dram_tensor("out", [1, OUT + 1], F32, kind="ExternalOutput")

    Sig = mybir.ActivationFunctionType.Sigmoid
    Tanh = mybir.ActivationFunctionType.Tanh

    with tile.TileContext(nc) as tc, ExitStack() as ctx:
        const = ctx.enter_context(tc.tile_pool(name="const", bufs=1))
        xpool = ctx.enter_context(tc.tile_pool(name="xp", bufs=6 * TC + 4))
        stp = ctx.enter_context(tc.tile_pool(name="stp", bufs=1, space="PSUM"))
        scanp = ctx.enter_context(tc.tile_pool(name="scanp", bufs=1, space="PSUM"))
        stsb = ctx.enter_context(tc.tile_pool(name="stsb", bufs=2))
        gip = ctx.enter_context(tc.tile_pool(name="gip", bufs=4, space="PSUM"))
        gitp = ctx.enter_context(tc.tile_pool(name="gitp", bufs=2))
        gtp = ctx.enter_context(tc.tile_pool(name="gtp", bufs=3))
        selp = ctx.enter_context(tc.tile_pool(name="selp", bufs=1))

        # hidden-state history: slot 0 = h_{-1} = 0, slot t+1 = h_t
        hs = const.tile([128, T + 1, NKC], DTW, tag="hs")
        nc.vector.memset(hs[:, 0, :], 0.0)
        warm = const.tile([1, 1], F32, tag="warm")
        nc.vector.memset(warm[:], 0.0)
        nc.scalar.activation(warm[:], warm[:], Sig)
        ones_sb = const.tile([128, 1], DTW, tag="ones")
        nc.vector.memset(ones_sb[:], 1.0)

        # ---- x-tile DMA (sync queue); chunk 0 queued before the weights ----
        def load_x(t):
            x0 = xpool.tile([128, C], DTW, tag="x", name="x0")
            nc.sync.dma_start(out=x0[:], in_=xb.ap()[0:128, t, :])
            x1 = xpool.tile([128, C], DTW, tag="x", name="x1")
            nc.sync.dma_start(out=x1[:], in_=xb.ap()[128:256, t, :])
            return x0, x1

        xtiles = {}
        for t in range(CHS[0]):
            xtiles[t] = load_x(t)

        # ---- persistent constants in SBUF (gpsimd DMA queue) ----
        # order matters: wih+aug gate GI(0); whh gates scan step 0; rest late
        wih_sb = []
        for kc in range(NKC):
            t_ = const.tile([128, G3], DTW, tag=f"wih{kc}", name=f"wih{kc}")
            nc.gpsimd.dma_start(out=t_[:], in_=wih.ap()[kc * 128:(kc + 1) * 128, :])
            wih_sb.append(t_)
        wih4_sb = const.tile([OUT + 1, G3], DTW, tag="wih4")
        nc.gpsimd.dma_start(out=wih4_sb[:], in_=wih.ap()[C:C + OUT + 1, :])
        aug_sb = const.tile([OUT + 1, T], DTW, tag="aug")
        nc.gpsimd.dma_start(out=aug_sb[:], in_=aug.ap())
        # whh split across both DMA queues to balance the startup fill
        whh_sb = []
        for kc in range(NKC):
            t_ = const.tile([128, G3], DTW, tag=f"whh{kc}", name=f"whh{kc}")
            eng = nc.gpsimd if kc >= 2 else nc.sync
            eng.dma_start(out=t_[:], in_=whh.ap()[kc * 128:(kc + 1) * 128, :])
            whh_sb.append(t_)
        bhn_sb = const.tile([NKC, 128], DTW, tag="bhn")
        nc.gpsimd.dma_start(out=bhn_sb[:], in_=bhn.ap())
        id4_sb = const.tile([NKC, NKC], DTW, tag="id4")
        nc.gpsimd.dma_start(out=id4_sb[:], in_=id4.ap())
        wfc_sb = const.tile([128, NKC * (OUT + 1)], F32, tag="wfc")
        nc.gpsimd.dma_start(out=wfc_sb[:], in_=wfc.ap())
        bfc_sb = const.tile([1, OUT + 1], F32, tag="bfc")
        nc.gpsimd.dma_start(out=bfc_sb[:], in_=bfc.ap())
        oh_sb = const.tile([128, T], F32, tag="oh")
        nc.gpsimd.dma_start(out=oh_sb[:], in_=oh.ap())

        st_ps = {}   # chunk -> psum tile [128, NKC, TC]
        st_sb = {}
        git_sb = {}

        def reduce_t(tcn, ti):
            """Accumulate sum over HW of x[:, t, :] into st_ps[tcn][:, :, ti]."""
            t = CH0[tcn] + ti
            if tcn not in st_ps:
                st_ps[tcn] = stp.tile([128, NKC, TC], F32, tag="st", name="st")
            x0, x1 = xtiles.pop(t)
            for kc in range(NKC):
                cs = slice(kc * 128, (kc + 1) * 128)
                nc.tensor.matmul(st_ps[tcn][:, kc, ti:ti + 1], x0[:, cs], ones_sb[:],
                                 start=True, stop=False)
                nc.tensor.matmul(st_ps[tcn][:, kc, ti:ti + 1], x1[:, cs], ones_sb[:],
                                 start=False, stop=True)

        def st_copy(tcn):
            sz = CHS[tcn]
            sts = stsb.tile([128, NKC, TC], DTW, tag="st", name="stsb")
            nc.scalar.copy(sts[:, :, 0:sz], st_ps.pop(tcn)[:, :, 0:sz])
            st_sb[tcn] = sts

        grow_sb = {}   # chunk -> [TC, 2*H] f16 rows (r,z gi values by timestep)
        gprep_sb = {}  # chunk -> [NKC, TC, 2, 128] f16 lhsT slices for preloads

        def gi_row(tcn, nch):
            """Rows-form GI for the r/z gates, N-chunk nch of 2 (512 cols)."""
            t0, sz = CH0[tcn], CHS[tcn]
            sts = st_sb[tcn]
            if tcn not in grow_sb:
                grow_sb[tcn] = stsb.tile([TC, 2 * H], F16, tag="grow", name="grow")
            js = slice(nch * 512, (nch + 1) * 512)
            gps = gip.tile([TC, 512], F32, tag="gir", name="gir", bufs=2)
            for kc in range(NKC):
                nc.tensor.matmul(gps[0:sz, :], sts[:, kc, 0:sz], wih_sb[kc][:, js],
                                 start=(kc == 0), stop=False)
            nc.tensor.matmul(gps[0:sz, :], aug_sb[:, t0:t0 + sz], wih4_sb[:, js],
                             start=False, stop=True)
            nc.scalar.copy(grow_sb[tcn][0:sz, js], gps[0:sz, :])

        def gi_dma(tcn, ti):
            """Scatter row ti of grow into [4,128] lhsT slices for r and z."""
            if tcn not in gprep_sb:
                gprep_sb[tcn] = stsb.tile([NKC, TC, 2, 128], F16, tag="gprep",
                                          name="gprep")
            gp = gprep_sb[tcn]
            grow = grow_sb[tcn]
            nc.gpsimd.dma_start(
                out=gp[:, ti, 0, :],
                in_=grow[ti:ti + 1, 0:H].rearrange("p (k m) -> p k m", k=NKC))
            nc.gpsimd.dma_start(
                out=gp[:, ti, 1, :],
                in_=grow[ti:ti + 1, H:2 * H].rearrange("p (k m) -> p k m", k=NKC))

        def gi_jg(tcn, jg):
            """Column-form GI for the n gate (j-groups 8..11)."""
            t0, sz = CH0[tcn], CHS[tcn]
            sts = st_sb[tcn]
            if tcn not in git_sb:
                git_sb[tcn] = gitp.tile([128, NJG, TC], F32, tag="git", name="git")
            git = git_sb[tcn]
            js = slice((2 * NKC + jg) * 128, (2 * NKC + jg + 1) * 128)
            gps = gip.tile([128, TC], F32, tag="gi", name="gi", bufs=2)
            for kc in range(NKC):
                nc.tensor.matmul(gps[:, 0:sz], wih_sb[kc][:, js], sts[:, kc, 0:sz],
                                 start=(kc == 0), stop=False)
            nc.tensor.matmul(gps[:, 0:sz], wih4_sb[:, js], aug_sb[:, t0:t0 + sz],
                             start=False, stop=True)
            nc.scalar.copy(git[:, jg, 0:sz], gps[:, 0:sz])

        def gi_chunk(tcn):
            st_copy(tcn)
            for nch in range(2):
                gi_row(tcn, nch)
            for jg in range(NKC):
                gi_jg(tcn, jg)
            for ti in range(CHS[tcn]):
                gi_dma(tcn, ti)

        def scan_step(tcn, ti):
            t = CH0[tcn] + ti
            git = git_sb[tcn]
            gp = gprep_sb[tcn]
            h_prev = hs[:, t, :]
            psr = scanp.tile([128, NKC], F32, tag="psr", name="psr")
            psn = scanp.tile([128, NKC], F32, tag="psn", name="psn")
            psz = scanp.tile([128, NKC], F32, tag="psz", name="psz")

            def gh(ps, g):
                for jg4 in range(NKC):
                    js = slice((g * NKC + jg4) * 128, (g * NKC + jg4 + 1) * 128)
                    for kc in range(NKC):
                        nc.tensor.matmul(ps[:, jg4:jg4 + 1], whh_sb[kc][:, js],
                                         h_prev[:, kc:kc + 1],
                                         start=False,
                                         stop=(kc == 3 and jg4 == 3),
                                         skip_group_check=True)

            # r group; gi_r preloaded via identity matmul, sigmoid reads psum
            nc.tensor.matmul(psr[:], gp[:, ti, 0, :], id4_sb[:], start=True,
                             stop=False, skip_group_check=True)
            gh(psr, 0)
            r_t = gtp.tile([128, NKC], F32, tag="r")
            nc.scalar.activation(r_t[:], psr[:], Sig)
            # n group: preload b_hh_n into psum, then accumulate W_hh rows
            nc.tensor.matmul(psn[:], bhn_sb[:], id4_sb[:], start=True, stop=False,
                             skip_group_check=True)
            gh(psn, 2)
            a_t = gtp.tile([128, NKC], F32, tag="a")
            nc.vector.tensor_mul(a_t[:], r_t[:], psn[:])
            pre_n = gtp.tile([128, NKC], F32, tag="pre_n")
            nc.vector.tensor_add(pre_n[:], a_t[:], git[:, 0:NKC, ti])
            n_t = gtp.tile([128, NKC], F32, tag="n")
            nc.scalar.activation(n_t[:], pre_n[:], Tanh)
            # z group; gi_z preloaded, sigmoid reads psum
            nc.tensor.matmul(psz[:], gp[:, ti, 1, :], id4_sb[:], start=True,
                             stop=False, skip_group_check=True)
            gh(psz, 1)
            z_t = gtp.tile([128, NKC], F32, tag="z")
            nc.scalar.activation(z_t[:], psz[:], Sig)
            # h' = n + z*(h - n); d computable during sigmoid(z)
            d_t = gtp.tile([128, NKC], F32, tag="d")
            nc.vector.tensor_sub(d_t[:], h_prev[:], n_t[:])
            e_t = gtp.tile([128, NKC], F32, tag="e")
            nc.vector.tensor_mul(e_t[:], z_t[:], d_t[:])
            nc.vector.tensor_add(hs[:, t + 1, :], n_t[:], e_t[:])

        # ---- pipeline: 2.5-deep ----
        # during chunk k's scan: GI(k+1) and reduce(k+2)+st(k+2) spread
        # across all of chunk k's steps, interleaved to avoid ACT bunching
        NCHV = len(CHS)
        for cc in range(min(2, NCHV - 1)):
            for t in range(CH0[cc + 1], CH0[cc + 1] + CHS[cc + 1]):
                xtiles[t] = load_x(t)
        for ti in range(CHS[0]):
            reduce_t(0, ti)
        gi_chunk(0)
        if NCHV > 1:
            for ti in range(CHS[1]):
                reduce_t(1, ti)
            st_copy(1)
        for tcn in range(NCHV):
            sz = CHS[tcn]
            if tcn + 3 < NCHV:
                for t in range(CH0[tcn + 3], CH0[tcn + 3] + CHS[tcn + 3]):
                    xtiles[t] = load_x(t)
            gis = (([("gr", tcn + 1, nch) for nch in range(2)]
                    + [("gi", tcn + 1, jg) for jg in range(NKC)]
                    + [("gd", tcn + 1, ti) for ti in range(CHS[tcn + 1])])
                   if tcn + 1 < NCHV else [])
            reds = ([("red", tcn + 2, ti) for ti in range(CHS[tcn + 2])]
                    + [("st", tcn + 2, 0)]) if tcn + 2 < NCHV else []
            units = []
            gper = -(-len(gis) // sz)
            rper = -(-len(reds) // sz) if reds else 0
            for ti in range(sz):
                step_units = []
                g = gis[ti * gper:(ti + 1) * gper]
                r = reds[ti * rper:(ti + 1) * rper]
                for k in range(max(len(g), len(r))):
                    if k < len(g):
                        step_units.append(g[k])
                    if k < len(r):
                        step_units.append(r[k])
                units.append(step_units)
            for ti in range(sz):
                scan_step(tcn, ti)
                for kind, a1, a2 in units[ti]:
                    if kind == "red":
                        reduce_t(a1, a2)
                    elif kind == "st":
                        st_copy(a1)
                    elif kind == "gr":
                        gi_row(a1, a2)
                    elif kind == "gd":
                        gi_dma(a1, a2)
                    else:
                        gi_jg(a1, a2)

        # ---- select h at t = ori_len-1 via one-hot, then final fc ----
        selt = selp.tile([128, NKC, T], F32, tag="selt")
        for kc in range(NKC):
            nc.vector.tensor_mul(selt[:, kc, :], hs[:, 1:T + 1, kc], oh_sb[:])
        hsel = selp.tile([128, NKC], F32, tag="hsel")
        nc.vector.tensor_reduce(hsel[:], selt[:], axis=mybir.AxisListType.X,
                                op=mybir.AluOpType.add)
        psfc = scanp.tile([1, OUT + 1], F32, tag="psr")
        for kc in range(NKC):
            nc.tensor.matmul(psfc[:], hsel[:, kc:kc + 1],
                             wfc_sb[:, kc * (OUT + 1):(kc + 1) * (OUT + 1)],
                             start=(kc == 0), stop=(kc == 3))
        out_sb = selp.tile([1, OUT + 1], F32, tag="osb")
        nc.vector.tensor_add(out_sb[:], psfc[:], bfc_sb[:])
        nc.sync.dma_start(out=outp.ap(), in_=out_sb[:])

    nc.compile()
    return nc


_NC_CACHE = None


def _get_nc():
    global _NC_CACHE
    if _NC_CACHE is None:
        _NC_CACHE = build_program()
    return _NC_CACHE


def _install_profile_hook():
    """Make run_bass_kernel_spmd(trace=True) work in this container."""
    if "antenv.axon_hooks" in sys.modules:
        return
    try:
        import trn_agent_boot.trn_boot as _boot
        hook = _boot._ntff_profile_via_ctypes("/opt/axon/libaxon_pjrt.so")
    except Exception:
        hook = None
    shim = types.ModuleType("antenv.axon_hooks")
    shim.get_axon_ntff_profile_hook = lambda: hook
    shim.set_axon_ntff_profile_hook = lambda h: None
    sys.modules["antenv.axon_hooks"] = shim


def make_in_maps(x, ori_len, gt, W_ih, W_hh, b_ih, b_hh, Wfc, bfc):
    x = np.asarray(x, np.float32)
    ori_len = np.asarray(ori_len)
    gt = np.asarray(gt, np.float32)
    W_ih = np.asarray(W_ih, np.float32)
    W_hh = np.asarray(W_hh, np.float32)
    b_ih = np.asarray(b_ih, np.float32)
    b_hh = np.asarray(b_hh, np.float32)
    Wfc = np.asarray(Wfc, np.float32)
    bfc = np.asarray(bfc, np.float32)

    wih_host = np.zeros((C + OUT + 1, G3), np.float32)
    wih_host[:C + OUT] = W_ih.T
    brz = b_ih + np.concatenate([b_hh[:2 * H], np.zeros(H, np.float32)])
    wih_host[C + OUT] = brz
    whh_host = np.ascontiguousarray(W_hh.T)
    bhn_host = np.ascontiguousarray(b_hh[2 * H:].reshape(NKC, 128)).astype(NPW)
    id4_host = np.eye(NKC, dtype=NPW)
    wfc_host = np.ascontiguousarray(
        Wfc.reshape(NKC, 128, OUT + 1).transpose(1, 0, 2).reshape(128, -1))
    bfc_host = bfc[None, :]
    idx = np.clip(ori_len.astype(np.int64) - 1, 0, T - 1)

    in_maps = []
    for b in range(B):
        aug_host = np.zeros((OUT + 1, T), np.float32)
        aug_host[:OUT] = gt[b][:, None]
        aug_host[OUT] = 1.0
        oh_host = np.zeros((T,), np.float32)
        oh_host[idx[b]] = 1.0
        in_maps.append({
            "xb": np.ascontiguousarray(x[b]).astype(NPW),
            "wih": wih_host.astype(NPW),
            "whh": whh_host.astype(NPW),
            "aug": aug_host.astype(NPW),
            "bhn": bhn_host,
            "id4": id4_host,
            "wfc": wfc_host,
            "bfc": bfc_host,
            "oh": np.repeat(oh_host[None, :], 128, axis=0),
        })
    return in_maps


def kernel(x, ori_len, gt, W1, b1, W2, b2, V, bV, W_ih, W_hh, b_ih, b_hh,
           Wfc, bfc, _trace=False):
    # Attention softmax is over a singleton dim -> weights == 1 exactly, so
    # ctx = x.sum(axis=1) and W1/b1/W2/b2/V/bV cannot affect the output.
    nc = _get_nc()
    in_maps = make_in_maps(x, ori_len, gt, W_ih, W_hh, b_ih, b_hh, Wfc, bfc)
    if _trace:
        _install_profile_hook()
    res = run_bass_kernel_spmd(nc, in_maps, list(range(B)), trace=_trace)
    out = np.stack([res.results[b]["out"][0] for b in range(B)]).astype(np.float32)
    if _trace:
        return out, res
    return out
